# revision 9
# baseline (speedup 1.0000x reference)
"""Bilateral grid slicing kernel v3 for Trainium2 (8 NeuronCores, SPMD).

Per pixel: z = gray(rgb); trilinear sample of (12, 8, 16, 16) grid at
(x, y, z); apply the resulting 3x4 affine to rgb.

v3 design: the v2 bottleneck was the gpsimd ap_gather (software memcpy on
the Q7 cores, ~63 GB/s, ~100us/tile). v3 replaces it with SWDGE
dma_gather: the Pool engine only *generates descriptors* (~0.34ns each)
and the 16 DMA engines execute the gather from an HBM table at full DMA
bandwidth. Table: one 256B line per cell (z0,y0,x0) holding all 8
corner values x 12 channels (f16, corner-clamping baked on host), so a
single descriptor fetches everything a pixel needs, landing in the
pixel's own partition (dst[s%128, s//128] = line[idx[s]]) - which also
eliminates v2's 96 PE transposes per tile.

- idx stream must be wrapped [16, n/16] + replicated for gpsimd cores
  0/1 (queue 0); built via a DRAM round-trip (SBUF has no partition-
  crossing DMA path).
- floors: q-0.5 folded into producer Act ops, then the 1.5*2^23
  float-magic round (pure f32 adds; identical on CoreSim and HW).
- Blend on DVE in f16 at 2 elem/cycle: one mult per gathered value
  (weights w8 = wz*wy*wx prebuilt), 3-level pair add-tree over corners,
  then the 3x4 affine as in v2. All APs <= 3D.
"""
import sys

sys.path.insert(0, "/opt/trn_rl_repo")
import numpy as np

import concourse.bass as bass
import concourse.bacc as bacc
import concourse.tile as tile
from concourse import mybir
from concourse import bass_utils

F32 = mybir.dt.float32
F16 = mybir.dt.float16
I16 = mybir.dt.int16
Alu = mybir.AluOpType
ActFn = mybir.ActivationFunctionType

N_CORES = 8
H, W = 1080, 1920
HH = H // 2                     # rows per core
P_CORE = HH * W                 # 1,036,800 pixels per core
T = 256                         # pixels per partition per tile
N_TILE = 128 * T                # 32768 pixels per tile
N_TILES = (P_CORE + N_TILE - 1) // N_TILE   # 32 (padded)
P_PAD = N_TILES * N_TILE        # 1,048,576

GL, GH, GW = 8, 16, 16
NCELL = GL * GH * GW            # 2048
LINE = 128                      # f16 elems per table line (96 used + pad)
GIDX = 1024                     # idxs per dma_gather call (SWDGE ring holds
                                # only ~65 descs/DMA on this runtime; 1024
                                # idxs -> 65 descs/DMA is the max that works)
GROUPS = 4                      # blend chunks per tile
SUBS = N_TILE // GROUPS // GIDX  # 8 gather calls per blend chunk
GQ = GROUPS and (N_TILE // GROUPS // 128)  # 64 q-columns per blend chunk

_cache = {}


def _ap(t, extra_dims, offset=0):
    """AP on tile t keeping partition dim, custom free dims (elem units)."""
    a = t[:] if not isinstance(t, bass.AP) else t
    return bass.AP(tensor=a.tensor, offset=a.offset + offset,
                   ap=[list(a.ap[0])] + [list(d) for d in extra_dims])


def _build(n_tiles):
    nc = bacc.Bacc("TRN2", target_bir_lowering=False)
    n_pix = N_TILES * N_TILE
    with tile.TileContext(nc) as tc:
        with tc.tile_pool(name="dram", bufs=1, space="DRAM") as dram:
            gxy = dram.tile([n_pix, 2], F32, kind="ExternalInput", name="gxy", uniquify=False)
            rgb = dram.tile([n_pix, 3], F32, kind="ExternalInput", name="rgb", uniquify=False)
            tab = dram.tile([NCELL, LINE], F16, kind="ExternalInput", name="tab", uniquify=False)
            out = dram.tile([n_pix, 3], F32, kind="ExternalOutput", name="out", uniquify=False)
            _body(nc, tc, n_tiles, gxy, rgb, tab, out)
    nc.compile()
    return nc


def _build_small(n_tiles):
    """Small-loop NEFF with FULL-size io tensors: used by test.py to measure
    the on-device time differentially (equal per-call transfer cost)."""
    return _build(n_tiles)


def _body(nc, tc, n_tiles, gxy, rgb, tab, out):
    import contextlib
    ctx = contextlib.ExitStack()
    io = ctx.enter_context(tc.tile_pool(name="io", bufs=2))
    wk = ctx.enter_context(tc.tile_pool(name="wk", bufs=2))
    gkp = ctx.enter_context(tc.tile_pool(name="gkp", bufs=3))
    mp = ctx.enter_context(tc.tile_pool(name="mp", bufs=2))
    ap_pool = ctx.enter_context(tc.tile_pool(name="ap", bufs=2))
    outp = ctx.enter_context(tc.tile_pool(name="outp", bufs=2))
    dramp = ctx.enter_context(tc.tile_pool(name="dramp", bufs=2, space="DRAM"))

    for it in range(n_tiles):
        j0 = it * N_TILE
        gxy_t = io.tile([128, T, 2], F32, tag="gxy_t")
        nc.sync.dma_start(out=gxy_t[:], in_=gxy[j0:j0 + N_TILE, :].rearrange("(p t) c -> p t c", p=128))
        rgb_t = io.tile([128, T, 3], F32, tag="rgb_t")
        nc.sync.dma_start(out=rgb_t[:], in_=rgb[j0:j0 + N_TILE, :].rearrange("(p t) c -> p t c", p=128))

        # iz = 7*gray(rgb) - 0.5 (the -0.5 folded into the last addend)
        iz = wk.tile([128, T], F32, tag="iz")
        nc.scalar.activation(iz[:], rgb_t[:, :, 0], ActFn.Copy, scale=0.299 * (GL - 1))
        zt1 = wk.tile([128, T], F32, tag="zt1")
        nc.scalar.activation(zt1[:], rgb_t[:, :, 1], ActFn.Copy, scale=0.587 * (GL - 1))
        zt2 = wk.tile([128, T], F32, tag="zt2")
        nc.scalar.activation(zt2[:], rgb_t[:, :, 2], ActFn.Copy, scale=0.114 * (GL - 1), bias=-0.5)
        nc.vector.tensor_tensor(iz[:], iz[:], zt1[:], Alu.add)
        nc.vector.tensor_tensor(iz[:], iz[:], zt2[:], Alu.add)

        # q2 = q - 0.5, then rne(q2) via the 1.5*2^23 float-magic round.
        ixf = wk.tile([128, T], F32, tag="ixf")
        nc.scalar.activation(ixf[:], gxy_t[:, :, 0], ActFn.Copy, scale=float(GW - 1), bias=-0.5)
        iyf = wk.tile([128, T], F32, tag="iyf")
        nc.scalar.activation(iyf[:], gxy_t[:, :, 1], ActFn.Copy, scale=float(GH - 1), bias=-0.5)

        def floor_of(q2, tag):
            qf = wk.tile([128, T], F32, tag=tag + "f")
            nc.scalar.activation(qf[:], q2[:], ActFn.Copy, bias=12582912.0)
            nc.scalar.activation(qf[:], qf[:], ActFn.Copy, bias=-12582912.0)
            return qf

        qfx = floor_of(ixf, "qx")
        qfy = floor_of(iyf, "qy")
        qfz = floor_of(iz, "qz")

        # fracs: s = q2 - floor = frac - 0.5; weight pairs (w0, w1) in f16
        sx = wk.tile([128, T], F32, tag="sx")
        nc.vector.tensor_tensor(sx[:], ixf[:], qfx[:], Alu.subtract)
        sy = wk.tile([128, T], F32, tag="sy")
        nc.vector.tensor_tensor(sy[:], iyf[:], qfy[:], Alu.subtract)
        sz = wk.tile([128, T], F32, tag="sz")
        nc.vector.tensor_tensor(sz[:], iz[:], qfz[:], Alu.subtract)
        wxp = wk.tile([128, T, 2], F16, tag="wxp")
        nc.scalar.activation(wxp[:, :, 0], sx[:], ActFn.Copy, scale=-1.0, bias=0.5)
        nc.scalar.activation(wxp[:, :, 1], sx[:], ActFn.Copy, bias=0.5)
        wyp = wk.tile([128, T, 2], F16, tag="wyp")
        nc.scalar.activation(wyp[:, :, 0], sy[:], ActFn.Copy, scale=-1.0, bias=0.5)
        nc.scalar.activation(wyp[:, :, 1], sy[:], ActFn.Copy, bias=0.5)
        wzp = wk.tile([128, T, 2], F16, tag="wzp")
        nc.scalar.activation(wzp[:, :, 0], sz[:], ActFn.Copy, scale=-1.0, bias=0.5)
        nc.scalar.activation(wzp[:, :, 1], sz[:], ActFn.Copy, bias=0.5)

        # v4[t, zy] = wz[dz] * wy[dy]   (zy = dz*2 + dy)
        v4 = wk.tile([128, T * 4], F16, tag="v4")
        nc.vector.tensor_tensor(
            _ap(v4, [[4, T], [2, 2], [1, 2]]),
            _ap(wzp, [[2, T], [1, 2], [0, 2]]),
            _ap(wyp, [[2, T], [0, 2], [1, 2]]),
            Alu.mult)
        # w8[t, corner] = v4[t, zy] * wx[dx]   (corner = zy*2 + dx)
        w8 = wk.tile([128, T * 8], F16, tag="w8")
        nc.vector.tensor_tensor(
            _ap(w8, [[8, T], [2, 4], [1, 2]]),
            _ap(v4, [[4, T], [1, 4], [0, 2]]),
            _ap(wxp, [[2, T], [0, 4], [1, 2]]),
            Alu.mult)

        # flat cell index -> int16 -> wrapped stream layout via DRAM
        idxf = wk.tile([128, T], F32, tag="idxf")
        nc.vector.scalar_tensor_tensor(idxf[:], qfz[:], float(GH), qfy[:], Alu.mult, Alu.add)
        nc.vector.scalar_tensor_tensor(idxf[:], idxf[:], float(GW), qfx[:], Alu.mult, Alu.add)
        idx16 = wk.tile([128, T], I16, tag="idx16")
        nc.scalar.activation(idx16[:], idxf[:], ActFn.Copy)
        idxd = dramp.tile([128, T], I16, tag="idxd")
        nc.sync.dma_start(out=idxd[:], in_=idx16[:])
        # wrapped[u, t*8 + r] = idx16[r*16 + u, t]; stream pos s = t*128 + p
        # lands pixel (p, t) at gather dst [p, t]. Replicated for cores 0, 1
        # (queue 0); partitions 32+ are unread on HW, zeroed for CoreSim.
        wrapped = wk.tile([128, T * 8], I16, tag="wrapped")
        nc.vector.memset(wrapped[:], 0)
        for g in range(2):
            nc.sync.dma_start(
                out=wrapped[16 * g:16 * (g + 1), :],
                in_=bass.AP(tensor=idxd.tensor, offset=idxd.offset,
                            ap=[[T, 16], [1, T], [16 * T, 8]]))

        # gather + blend, 4 chunks of 8192 pixels.
        # Table line layout v = c*8 + corner keeps every hot DVE op at the
        # 2x f16 mode (step-1 inner APs; the w8 broadcast is a stride-0
        # MIDDLE dim, which preserves packing). Only the final 2->1 corner
        # add (step-2 reads) and the small w8/v4 builds run at 1x.
        A = ap_pool.tile([128, T * 12], F16, tag="A")   # elem = t*12 + c
        for cchunk in range(GROUPS):
            gk = gkp.tile([128, GQ, LINE], F16, tag="gk")
            for sub in range(SUBS):
                call = cchunk * SUBS + sub
                nc.gpsimd.dma_gather(
                    out_ap=gk[:, sub * (GIDX // 128):(sub + 1) * (GIDX // 128), :],
                    in_ap=tab[:],
                    idxs_ap=wrapped[:, call * (GIDX // 16):(call + 1) * (GIDX // 16)],
                    num_idxs=GIDX,
                    num_idxs_reg=GIDX,
                    elem_size=LINE,
                )
            t0 = cchunk * GQ
            # m[t', c, corner] = gk * w8 (broadcast over c)
            m = mp.tile([128, GQ * 96], F16, tag="m")
            nc.vector.tensor_tensor(
                _ap(m, [[96, GQ], [8, 12], [1, 8]]),
                _ap(gk, [[LINE, GQ], [8, 12], [1, 8]]),
                _ap(w8, [[8, GQ], [0, 12], [1, 8]], offset=t0 * 8),
                Alu.mult)
            # halves add-tree over the corner bits: dz, then dy, then dx
            r1 = mp.tile([128, GQ * 48], F16, tag="r1")
            nc.vector.tensor_tensor(
                _ap(r1, [[48, GQ], [4, 12], [1, 4]]),
                _ap(m, [[96, GQ], [8, 12], [1, 4]]),
                _ap(m, [[96, GQ], [8, 12], [1, 4]], offset=4),
                Alu.add)
            r2 = mp.tile([128, GQ * 24], F16, tag="r2")
            nc.vector.tensor_tensor(
                _ap(r2, [[24, GQ], [2, 12], [1, 2]]),
                _ap(r1, [[48, GQ], [4, 12], [1, 2]]),
                _ap(r1, [[48, GQ], [4, 12], [1, 2]], offset=2),
                Alu.add)
            nc.vector.tensor_tensor(
                _ap(A, [[12, GQ], [1, 12]], offset=t0 * 12),
                _ap(r2, [[24, GQ], [2, 12]]),
                _ap(r2, [[24, GQ], [2, 12]], offset=1),
                Alu.add)

        # affine: out_i = sum_j A[t, i*4+j] * rgbw[t, j]
        rgbw = wk.tile([128, T, 4], F16, tag="rgbw")
        nc.scalar.activation(rgbw[:, :, 0:3], rgb_t[:], ActFn.Copy)
        nc.vector.memset(rgbw[:, :, 3], 1.0)
        m2 = outp.tile([128, T * 12], F16, tag="m2")    # (t, i, j)
        nc.vector.tensor_tensor(
            _ap(m2, [[12, T], [4, 3], [1, 4]]),
            _ap(A, [[12, T], [4, 3], [1, 4]]),
            _ap(rgbw, [[4, T], [0, 3], [1, 4]]),
            Alu.mult)
        mm1 = outp.tile([128, T * 6], F16, tag="mm1")   # (t, i, j2)
        nc.vector.tensor_tensor(
            _ap(mm1, [[6, T], [2, 3], [1, 2]]),
            _ap(m2, [[12, T], [4, 3], [1, 2]]),
            _ap(m2, [[12, T], [4, 3], [1, 2]], offset=2),
            Alu.add)
        o3 = outp.tile([128, T * 3], F16, tag="o3")     # (t, i)
        nc.vector.tensor_tensor(
            _ap(o3, [[3, T], [1, 3]]),
            _ap(mm1, [[6, T], [2, 3]]),
            _ap(mm1, [[6, T], [2, 3]], offset=1),
            Alu.add)
        outf = outp.tile([128, T * 3], F32, tag="outf")
        nc.scalar.activation(outf[:], o3[:], ActFn.Copy)
        nc.sync.dma_start(
            out=bass.AP(tensor=out.tensor, offset=out.offset + j0 * 3,
                        ap=[[T * 3, 128], [1, T * 3]]),
            in_=outf[:])
    ctx.close()


def _pack_tables(grids_view):
    """grids_view: (12, 8, 16, 16) f32 -> [2048, 128] f16 corner lines.

    line[cell=(z0,y0,x0)][c*8 + corner] = g[c, z0+dz|, y0+dy|, x0+dx|]
    with corner = dz*4 + dy*2 + dx and | = clamp; cols 96..127 are pad.
    """
    g = grids_view.astype(np.float32)  # (12, 8, 16, 16)
    z = np.arange(GL)[:, None, None]
    y = np.arange(GH)[None, :, None]
    x = np.arange(GW)[None, None, :]
    tabs = np.zeros((NCELL, LINE), dtype=np.float16)
    for dz in range(2):
        zz = np.minimum(z + dz, GL - 1)
        for dy in range(2):
            yy = np.minimum(y + dy, GH - 1)
            for dx in range(2):
                xx = np.minimum(x + dx, GW - 1)
                corner = dz * 4 + dy * 2 + dx
                # broadcasts to (12, 8, 16, 16)
                v = g[:, zz, yy, np.broadcast_to(xx, (GL, GH, GW))]
                flat = v.reshape(12, NCELL)          # cell = z*256+y*16+x
                tabs[:, corner:96:8] = flat.T.astype(np.float16)
    return tabs


def _shards(grid_xy, rgb, grids):
    """Split full inputs into 8 per-core input maps (padded)."""
    maps = []
    for k in range(N_CORES):
        vv, hh = k // 2, k % 2
        gxy_s = grid_xy[vv, 0, hh * HH:(hh + 1) * HH].reshape(-1, 2)
        rgb_s = rgb[vv, 0, hh * HH:(hh + 1) * HH].reshape(-1, 3)
        pad = P_PAD - P_CORE
        gxy_s = np.concatenate([gxy_s, np.zeros((pad, 2), np.float32)])
        rgb_s = np.concatenate([rgb_s, np.zeros((pad, 3), np.float32)])
        maps.append({
            "gxy": np.ascontiguousarray(gxy_s),
            "rgb": np.ascontiguousarray(rgb_s),
            "tab": _pack_tables(grids[vv]),
        })
    return maps


def kernel(grid_xy, rgb, grids):
    if "nc" not in _cache:
        _cache["nc"] = _build(N_TILES)
    nc = _cache["nc"]
    maps = _shards(grid_xy, rgb, grids)
    res = bass_utils.run_bass_kernel_spmd(nc, maps, core_ids=list(range(N_CORES)))
    outv = np.empty((4, 1, H, W, 3), np.float32)
    for k in range(N_CORES):
        vv, hh = k // 2, k % 2
        o = res.results[k]["out"][:P_CORE].reshape(HH, W, 3)
        outv[vv, 0, hh * HH:(hh + 1) * HH] = o
    return outv
